# revision 19
# baseline (speedup 1.0000x reference)
"""Causal self-attention (B=2, T=4096, C=768, H=12) on 8 trn2 NeuronCores.

Sharding: core c handles batch b = c//4 and the 3 heads of head-group
hg = c%4 (tensor parallel over heads, data parallel over batch).  Each core
computes the qkv projection for its heads, causal attention, and a partial
output projection; the host sums the 4 per-head-group partials per batch.

Device notes:
  - Matmul inputs are bf16 (fp32 matmul runs LOW_HIGH = 2 PE passes);
    accumulation is fp32 in PSUM.  Host pre-transposes all operands so the
    contraction dim is on SBUF partitions.
  - Scores are computed transposed (S^T[tk, tq] = K Q^T) so P V needs no
    on-chip transposes.  The d=64 contraction uses 2x row tiling: two heads
    (partitions 0-63 / 64-127) run concurrently in the 64x128 PE mode.
    Attention is emitted in batched stages (all S^T, then all PV per group)
    to minimize PE tiling-mode switches.
  - Softmax denominator comes from an all-ones 65th column appended to V;
    normalization broadcasts the reciprocal row across partitions on GpSimd.
    Softmax skips the max subtraction: scores are ~N(0,1), exp is fp32-safe.
"""

import ml_dtypes
import numpy as np

import concourse.bass as bass
import concourse.mybir as mybir
import concourse.tile as tile
from concourse import bacc

B, T, C, H, HD = 2, 4096, 768, 12, 64
F32 = mybir.dt.float32
BF16 = mybir.dt.bfloat16
N_CORES = 8
AF = mybir.ActivationFunctionType


def build_nc(seq_len: int = T) -> bass.Bass:
    assert seq_len % 512 == 0
    TCH = seq_len // 512   # 512-wide t-chunks
    TB = seq_len // 128    # 128-wide t-blocks

    nc = bacc.Bacc(num_devices=N_CORES)

    xT = nc.dram_tensor("xT", (C, seq_len), BF16, kind="ExternalInput").ap()
    wqkT = nc.dram_tensor("wqkT", (C, 384), BF16, kind="ExternalInput").ap()
    wvT = nc.dram_tensor("wvT", (C, 192), BF16, kind="ExternalInput").ap()
    wpT = nc.dram_tensor("wpT", (192, C), BF16, kind="ExternalInput").ap()
    out = nc.dram_tensor("out", (seq_len, C), F32, kind="ExternalOutput").ap()

    with tile.TileContext(nc) as tc:
        with (
            tc.tile_pool(name="const", bufs=1) as const,
            tc.tile_pool(name="persist", bufs=1) as persist,
            tc.tile_pool(name="xt", bufs=2) as xtpool,
            tc.tile_pool(name="p", bufs=24) as ppool,
            tc.tile_pool(name="small", bufs=4) as spool,
            tc.tile_pool(name="osb", bufs=3) as osbpool,
            tc.tile_pool(name="ps", bufs=6, space="PSUM") as pspool,
            tc.tile_pool(name="pso", bufs=2, space="PSUM") as psopool,
        ):
            # ---- constants / weights ----
            wqk_sb = const.tile([128, 6, 384], BF16, tag="wqk")
            nc.sync.dma_start(wqk_sb, wqkT.rearrange("(cc p) o -> p cc o", p=128))
            wv_sb = const.tile([128, 6, 192], BF16, tag="wv")
            nc.sync.dma_start(wv_sb, wvT.rearrange("(cc p) o -> p cc o", p=128))
            wp0_sb = const.tile([128, 768], BF16, tag="wp0")
            nc.sync.dma_start(wp0_sb, wpT[0:128, :])
            # zero-pad wp1 to 128 partitions so the proj matmul stays K=128
            # (avoids a 64x128 <-> 128x128 PE mode switch per t-block)
            wp1_sb = const.tile([128, 768], BF16, tag="wp1")
            nc.vector.memset(wp1_sb[64:128, :], 0.0)
            nc.sync.dma_start(wp1_sb[0:64, :], wpT[128:192, :])

            # emask[i, j] = 1.0 if j >= i + 384 else 0.0 (causal masks for the
            # 4 partially-masked k-blocks of each 512-wide q-chunk)
            emask = const.tile([128, 896], BF16, tag="emask")
            nc.gpsimd.memset(emask, 1.0)
            nc.gpsimd.affine_select(
                out=emask, in_=emask,
                compare_op=mybir.AluOpType.is_ge,
                fill=0.0, base=-384, pattern=[[1, 896]], channel_multiplier=-1,
            )

            # ---- persistent activations ----
            # qT/kT slab0: h0 @ partitions 0-63, h1 @ 64-127.
            # slab1: h2 duplicated to both halves (enables 2x row tiling
            # with h2 paired against itself on two q-chunks).
            qT_sb = persist.tile([128, 2, seq_len], BF16, tag="qT")
            kT_sb = persist.tile([128, 2, seq_len], BF16, tag="kT")
            # v per head: [t-partition, kb, 64 dims + ones column]
            v_sb = [
                persist.tile([128, TB, 65], BF16, tag=f"v{h}", name=f"v{h}")
                for h in range(3)
            ]
            for h in range(3):
                nc.gpsimd.memset(v_sb[h][:, :, 64], 1.0)
            # attention output, transposed: chunk0 = [h0 | h1], chunk1 = [h2 | 0]
            outT_sb = persist.tile([128, 2, seq_len], BF16, tag="outT")
            nc.vector.memset(outT_sb[64:128, 1, :], 0.0)

            # ---- phase 1: qkv projection ----
            for tci in range(TCH):
                tcs = slice(tci * 512, (tci + 1) * 512)
                xt = xtpool.tile([128, 6, 512], BF16, tag="xt")
                nc.sync.dma_start(
                    xt, xT[:, tcs].rearrange("(cc p) t -> p cc t", p=128)
                )
                # q/k channels: m0=[q_h0|q_h1], m1=[k_h0|k_h1], m2=[q_h2|k_h2]
                for m in range(3):
                    ps = pspool.tile([128, 512], F32, tag="ps")
                    for cc in range(6):
                        nc.tensor.matmul(
                            ps,
                            lhsT=wqk_sb[:, cc, m * 128:(m + 1) * 128],
                            rhs=xt[:, cc, :],
                            start=(cc == 0), stop=(cc == 5),
                        )
                    if m == 0:
                        nc.vector.tensor_copy(qT_sb[:, 0, tcs], ps[:, :512])
                    elif m == 1:
                        nc.vector.tensor_copy(kT_sb[:, 0, tcs], ps[:, :512])
                    else:
                        # h2: land q at 0-63 / k at 64-127, then duplicate to
                        # the opposite half via SBUF->SBUF DMA.
                        nc.vector.tensor_copy(qT_sb[0:64, 1, tcs], ps[0:64, :512])
                        nc.vector.tensor_copy(kT_sb[64:128, 1, tcs], ps[64:128, :512])
                        nc.sync.dma_start(qT_sb[64:128, 1, tcs], qT_sb[0:64, 1, tcs])
                        nc.sync.dma_start(kT_sb[0:64, 1, tcs], kT_sb[64:128, 1, tcs])
                # v channels
                for tb in range(4):
                    psv = pspool.tile([128, 512], F32, tag="ps")
                    for cc in range(6):
                        nc.tensor.matmul(
                            psv[:, :192],
                            lhsT=xt[:, cc, tb * 128:(tb + 1) * 128],
                            rhs=wv_sb[:, cc, :],
                            start=(cc == 0), stop=(cc == 5),
                        )
                    for h in range(3):
                        nc.vector.tensor_copy(
                            v_sb[h][:, tci * 4 + tb, 0:64],
                            psv[:, 64 * h:64 * h + 64],
                        )

            # ---- phase 2: attention ----
            # groups: (vA, qcA, vB, qcB) with the A job on partitions 0-63 of
            # slab `slabA` and B on 64-127 of `slabB`.
            groups = []
            for qc in range(TCH):
                groups.append((0, 0, qc, 1, 0, qc))        # h0 paired with h1
            for i in range((TCH + 1) // 2):
                qcb = 2 * i + 1 if 2 * i + 1 < TCH else None
                groups.append((2, 1, 2 * i, 2, 1, qcb))    # h2 with itself

            for hA, slabA, qcA, hB, slabB, qcB in groups:
                nkbA = 4 * (qcA + 1)
                nkbB = 4 * (qcB + 1) if qcB is not None else 0
                nkb_max = max(nkbA, nkbB)
                qcsA = slice(qcA * 512, (qcA + 1) * 512)
                qcsB = (
                    slice(qcB * 512, (qcB + 1) * 512) if qcB is not None else None
                )
                sides = [(nkbA, slabA, 0, qcsA, hA, qcA)]
                if qcB is not None:
                    sides.append((nkbB, slabB, 64, qcsB, hB, qcB))
                psos = [
                    psopool.tile([128, 512], F32, tag="pso", name=f"pso{s}")
                    for s in range(len(sides))
                ]

                # stage A: all S^T blocks (64x128 row-tiled mode; side A on
                # array rows 0-63, side B on 64-127, running concurrently),
                # exp'd per block as they drain.
                pts = [[] for _ in sides]  # per-side list of [128,512] P tiles
                for kb in range(nkb_max):
                    for side, (nkb, slab, base, qcs, _h, qc) in enumerate(sides):
                        if kb >= nkb:
                            continue
                        sp = pspool.tile([128, 512], F32, tag="ps", name="sp")
                        nc.tensor.matmul(
                            sp,
                            lhsT=kT_sb[base:base + 64, slab,
                                       kb * 128:(kb + 1) * 128],
                            rhs=qT_sb[base:base + 64, slab, qcs],
                            start=True, stop=True,
                        )
                        pt = ppool.tile([128, 512], BF16, tag="p", name="pt")
                        nc.scalar.activation(pt, sp, AF.Exp, scale=0.125)
                        poff = kb - 4 * qc
                        if poff >= 0:  # partially-causal diagonal block
                            nc.vector.tensor_mul(
                                pt, pt,
                                emask[:, 384 - 128 * poff: 896 - 128 * poff],
                            )
                        pts[side].append(pt)

                # stage B: all PV blocks (128x128 mode)
                for kb in range(nkb_max):
                    for side, (nkb, slab, base, qcs, h, _qc) in enumerate(sides):
                        if kb >= nkb:
                            continue
                        nc.tensor.matmul(
                            psos[side][0:65, :],
                            lhsT=v_sb[h][:, kb, :],
                            rhs=pts[side][kb],
                            start=(kb == 0), stop=(kb == nkb - 1),
                        )

                # normalize: divide rows 0-63 by the ones-column row (64).
                # First evacuate PSUM to SBUF with one quick copy so the
                # PSUM bank frees immediately; the slow reciprocal chain then
                # runs off the critical path.
                for side, (nkb, slab, base, qcs, h, qc) in enumerate(sides):
                    pso = psos[side]
                    ocp = spool.tile([128, 512], F32, tag="ocp")
                    nc.vector.tensor_copy(ocp[0:65, :], pso[0:65, :])
                    rec = spool.tile([1, 512], F32, tag="rec")
                    nc.vector.reciprocal(rec, ocp[64:65, :])
                    bc = spool.tile([64, 512], F32, tag="bc")
                    nc.gpsimd.partition_broadcast(bc, rec)
                    if h == 1:
                        stg2 = spool.tile([64, 512], BF16, tag="stg2")
                        nc.vector.tensor_mul(stg2, ocp[0:64, :], bc)
                        nc.sync.dma_start(outT_sb[64:128, 0, qcs], stg2)
                    else:
                        nc.vector.tensor_mul(
                            outT_sb[0:64, slab, qcs], ocp[0:64, :], bc
                        )

            # ---- phase 3: output projection (partial over this core's heads) ----
            for tb in range(TB):
                tbs = slice(tb * 128, (tb + 1) * 128)
                ob = osbpool.tile([128, 768], F32, tag="osb")
                for n0, nsz in ((0, 512), (512, 256)):
                    pp = pspool.tile([128, 512], F32, tag="ps")
                    nc.tensor.matmul(
                        pp[:, :nsz],
                        lhsT=outT_sb[:, 0, tbs],
                        rhs=wp0_sb[:, n0:n0 + nsz],
                        start=True, stop=False,
                    )
                    nc.tensor.matmul(
                        pp[:, :nsz],
                        lhsT=outT_sb[:, 1, tbs],
                        rhs=wp1_sb[:, n0:n0 + nsz],
                        start=False, stop=True,
                    )
                    nc.vector.tensor_copy(ob[:, n0:n0 + nsz], pp[:, :nsz])
                nc.sync.dma_start(out[tbs, :], ob)

    nc.compile()
    return nc


_NC_CACHE: dict[int, bass.Bass] = {}


def get_nc(seq_len: int) -> bass.Bass:
    if seq_len not in _NC_CACHE:
        _NC_CACHE[seq_len] = build_nc(seq_len)
    return _NC_CACHE[seq_len]


def make_in_maps(x: np.ndarray, w_attn: np.ndarray, w_proj: np.ndarray):
    """Per-core input dicts. Core c: batch c//4, head group c%4 (3 heads)."""
    bf16 = ml_dtypes.bfloat16
    in_maps = []
    for c in range(N_CORES):
        b, hg = divmod(c, 4)
        q = w_attn[192 * hg: 192 * hg + 192]
        k = w_attn[768 + 192 * hg: 768 + 192 * hg + 192]
        v = w_attn[1536 + 192 * hg: 1536 + 192 * hg + 192]
        wqk = np.concatenate([q[0:128], k[0:128], q[128:192], k[128:192]], axis=0)
        in_maps.append({
            "xT": np.ascontiguousarray(x[b].T).astype(bf16),
            "wqkT": np.ascontiguousarray(wqk.T).astype(bf16),
            "wvT": np.ascontiguousarray(v.T).astype(bf16),
            "wpT": np.ascontiguousarray(
                w_proj[:, 192 * hg: 192 * hg + 192].T
            ).astype(bf16),
        })
    return in_maps


def run_on_cores(x, w_attn, w_proj, trace: bool = False):
    from concourse.bass_utils import run_bass_kernel_spmd

    x = np.asarray(x, dtype=np.float32)
    w_attn = np.asarray(w_attn, dtype=np.float32)
    w_proj = np.asarray(w_proj, dtype=np.float32)
    nc = get_nc(x.shape[1])
    in_maps = make_in_maps(x, w_attn, w_proj)
    res = run_bass_kernel_spmd(
        nc, in_maps, core_ids=list(range(N_CORES)), trace=trace
    )
    outs = [r["out"] for r in res.results]
    full = np.stack(
        [sum(outs[4 * b + hg] for hg in range(4)) for b in range(B)], axis=0
    )
    return full, res


def kernel(x, w_attn, w_proj):
    full, _ = run_on_cores(x, w_attn, w_proj, trace=False)
    return full


# revision 20
# speedup vs baseline: 1.2749x; 1.2749x over previous
"""Causal self-attention (B=2, T=4096, C=768, H=12) on 8 trn2 NeuronCores.

Sharding: core c handles batch b = c//4 and the 3 heads of head-group
hg = c%4 (tensor parallel over heads, data parallel over batch).  Each core
computes the qkv projection for its heads, causal attention, and a partial
output projection; the host sums the 4 per-head-group partials per batch.

Device notes:
  - Matmul inputs are bf16 (fp32 matmul runs LOW_HIGH = 2 PE passes);
    accumulation is fp32 in PSUM.  Host pre-transposes all operands so the
    contraction dim is on SBUF partitions.
  - Scores are computed transposed (S^T[tk, tq] = K Q^T) so P V needs no
    on-chip transposes.  The d=64 contraction uses 2x row tiling: two heads
    (partitions 0-63 / 64-127) run concurrently in the 64x128 PE mode.
    Attention is emitted in batched stages (all S^T, then all PV per group)
    to minimize PE tiling-mode switches.
  - Softmax denominator comes from an all-ones 65th column appended to V;
    normalization broadcasts the reciprocal row across partitions on GpSimd.
    Softmax skips the max subtraction: scores are ~N(0,1), exp is fp32-safe.
"""

import ml_dtypes
import numpy as np

import concourse.bass as bass
import concourse.mybir as mybir
import concourse.tile as tile
from concourse import bacc

B, T, C, H, HD = 2, 4096, 768, 12, 64
F32 = mybir.dt.float32
BF16 = mybir.dt.bfloat16
N_CORES = 8
AF = mybir.ActivationFunctionType


def build_nc(seq_len: int = T) -> bass.Bass:
    assert seq_len % 512 == 0
    TCH = seq_len // 512   # 512-wide t-chunks
    TB = seq_len // 128    # 128-wide t-blocks

    nc = bacc.Bacc(num_devices=N_CORES)

    xT = nc.dram_tensor("xT", (C, seq_len), BF16, kind="ExternalInput").ap()
    wqkT = nc.dram_tensor("wqkT", (C, 384), BF16, kind="ExternalInput").ap()
    wvT = nc.dram_tensor("wvT", (C, 192), BF16, kind="ExternalInput").ap()
    wpT = nc.dram_tensor("wpT", (192, C), BF16, kind="ExternalInput").ap()
    out = nc.dram_tensor("out", (seq_len, C), F32, kind="ExternalOutput").ap()

    with tile.TileContext(nc) as tc:
        with (
            tc.tile_pool(name="const", bufs=1) as const,
            tc.tile_pool(name="persist", bufs=1) as persist,
            tc.tile_pool(name="xt", bufs=2) as xtpool,
            tc.tile_pool(name="p", bufs=24) as ppool,
            tc.tile_pool(name="small", bufs=4) as spool,
            tc.tile_pool(name="osb", bufs=3) as osbpool,
            tc.tile_pool(name="ps", bufs=2, space="PSUM") as pspool,
            tc.tile_pool(name="pso", bufs=3, space="PSUM") as psopool,
            tc.tile_pool(name="pp", bufs=1, space="PSUM") as psppool,
        ):
            # ---- constants / weights ----
            wqk_sb = const.tile([128, 6, 384], BF16, tag="wqk")
            nc.sync.dma_start(wqk_sb, wqkT.rearrange("(cc p) o -> p cc o", p=128))
            wv_sb = const.tile([128, 6, 192], BF16, tag="wv")
            nc.sync.dma_start(wv_sb, wvT.rearrange("(cc p) o -> p cc o", p=128))
            wp0_sb = const.tile([128, 768], BF16, tag="wp0")
            nc.sync.dma_start(wp0_sb, wpT[0:128, :])
            # zero-pad wp1 to 128 partitions so the proj matmul stays K=128
            # (avoids a 64x128 <-> 128x128 PE mode switch per t-block)
            wp1_sb = const.tile([128, 768], BF16, tag="wp1")
            nc.vector.memset(wp1_sb[64:128, :], 0.0)
            nc.sync.dma_start(wp1_sb[0:64, :], wpT[128:192, :])

            # emask[i, j] = 1.0 if j >= i + 384 else 0.0 (causal masks for the
            # 4 partially-masked k-blocks of each 512-wide q-chunk)
            emask = const.tile([128, 896], BF16, tag="emask")
            nc.gpsimd.memset(emask, 1.0)
            nc.gpsimd.affine_select(
                out=emask, in_=emask,
                compare_op=mybir.AluOpType.is_ge,
                fill=0.0, base=-384, pattern=[[1, 896]], channel_multiplier=-1,
            )

            # ---- persistent activations ----
            # qT/kT slab0: h0 @ partitions 0-63, h1 @ 64-127.
            # slab1: h2 duplicated to both halves (enables 2x row tiling
            # with h2 paired against itself on two q-chunks).
            qT_sb = persist.tile([128, 2, seq_len], BF16, tag="qT")
            kT_sb = persist.tile([128, 2, seq_len], BF16, tag="kT")
            # v per head: [t-partition, kb, 64 dims + ones column]
            v_sb = [
                persist.tile([128, TB, 65], BF16, tag=f"v{h}", name=f"v{h}")
                for h in range(3)
            ]
            for h in range(3):
                nc.gpsimd.memset(v_sb[h][:, :, 64], 1.0)
            # attention output, transposed: chunk0 = [h0 | h1], chunk1 = [h2 | 0]
            outT_sb = persist.tile([128, 2, seq_len], BF16, tag="outT")
            nc.vector.memset(outT_sb[64:128, 1, :], 0.0)

            # ---- phase 1: qkv projection ----
            for tci in range(TCH):
                tcs = slice(tci * 512, (tci + 1) * 512)
                xt = xtpool.tile([128, 6, 512], BF16, tag="xt")
                nc.sync.dma_start(
                    xt, xT[:, tcs].rearrange("(cc p) t -> p cc t", p=128)
                )
                # q/k channels: m0=[q_h0|q_h1], m1=[k_h0|k_h1], m2=[q_h2|k_h2]
                for m in range(3):
                    ps = pspool.tile([128, 512], F32, tag="ps")
                    for cc in range(6):
                        nc.tensor.matmul(
                            ps,
                            lhsT=wqk_sb[:, cc, m * 128:(m + 1) * 128],
                            rhs=xt[:, cc, :],
                            start=(cc == 0), stop=(cc == 5),
                        )
                    if m == 0:
                        nc.vector.tensor_copy(qT_sb[:, 0, tcs], ps[:, :512])
                    elif m == 1:
                        nc.vector.tensor_copy(kT_sb[:, 0, tcs], ps[:, :512])
                    else:
                        # h2: land q at 0-63 / k at 64-127, then duplicate to
                        # the opposite half via SBUF->SBUF DMA.
                        nc.vector.tensor_copy(qT_sb[0:64, 1, tcs], ps[0:64, :512])
                        nc.vector.tensor_copy(kT_sb[64:128, 1, tcs], ps[64:128, :512])
                        nc.sync.dma_start(qT_sb[64:128, 1, tcs], qT_sb[0:64, 1, tcs])
                        nc.sync.dma_start(kT_sb[0:64, 1, tcs], kT_sb[64:128, 1, tcs])
                # v channels
                for tb in range(4):
                    psv = pspool.tile([128, 512], F32, tag="ps")
                    for cc in range(6):
                        nc.tensor.matmul(
                            psv[:, :192],
                            lhsT=xt[:, cc, tb * 128:(tb + 1) * 128],
                            rhs=wv_sb[:, cc, :],
                            start=(cc == 0), stop=(cc == 5),
                        )
                    for h in range(3):
                        nc.vector.tensor_copy(
                            v_sb[h][:, tci * 4 + tb, 0:64],
                            psv[:, 64 * h:64 * h + 64],
                        )

            # ---- phase 2 + 3: attention, fused with output projection ----
            # Per q-chunk: h0/h1 are row-tile paired (array rows 0-63 vs
            # 64-127); h2 pairs with itself by splitting its k-block range
            # across the duplicated slab halves.  All S^T matmuls run in the
            # 64x128 row-tiled mode, all PV matmuls in 128x128 mode, and the
            # projection for the finished q-chunk fills PE gaps.
            for qc in range(TCH):
                qcs = slice(qc * 512, (qc + 1) * 512)
                nkb = 4 * (qc + 1)
                half = nkb // 2
                ptmap = {}

                def st_pair(sides, qc=qc, qcs=qcs, ptmap=ptmap):
                    # sides: (head, slab, base, kb0) x2 -> one [128,1024] psum
                    # per side holding k-blocks kb0, kb0+1
                    sps = []
                    for h, slab, base, kb0 in sides:
                        sp = pspool.tile([128, 1024], F32, tag="ps", name="sp")
                        sps.append(sp)
                    for t in (0, 1):
                        for (h, slab, base, kb0), sp in zip(sides, sps):
                            kb = kb0 + t
                            nc.tensor.matmul(
                                sp[:, t * 512:(t + 1) * 512],
                                lhsT=kT_sb[base:base + 64, slab,
                                           kb * 128:(kb + 1) * 128],
                                rhs=qT_sb[base:base + 64, slab, qcs],
                                start=True, stop=True,
                            )
                    for (h, slab, base, kb0), sp in zip(sides, sps):
                        pt = ppool.tile([128, 1024], BF16, tag="p", name="pt")
                        nc.scalar.activation(pt, sp, AF.Exp, scale=0.125)
                        for t in (0, 1):
                            kb = kb0 + t
                            hs = slice(t * 512, (t + 1) * 512)
                            poff = kb - 4 * qc
                            if poff >= 0:  # partially-causal diagonal block
                                nc.vector.tensor_mul(
                                    pt[:, hs], pt[:, hs],
                                    emask[:, 384 - 128 * poff: 896 - 128 * poff],
                                )
                            ptmap[(h, kb)] = (pt, hs)

                # stage A: S^T + exp + mask
                for kbp in range(0, nkb, 2):
                    st_pair(((0, 0, 0, kbp), (1, 0, 64, kbp)))
                for j in range(0, half, 2):
                    st_pair(((2, 1, 0, j), (2, 1, 64, half + j)))

                # stage B: PV accumulation per head, then normalize
                for h, slab in ((0, 0), (1, 0), (2, 1)):
                    pso = psopool.tile([128, 512], F32, tag="pso", name="pso")
                    for kb in range(nkb):
                        pt, hs = ptmap[(h, kb)]
                        nc.tensor.matmul(
                            pso[0:65, :],
                            lhsT=v_sb[h][:, kb, :],
                            rhs=pt[:, hs],
                            start=(kb == 0), stop=(kb == nkb - 1),
                        )
                    # evacuate PSUM on ACT so the bank frees quickly; the
                    # slow reciprocal chain then runs off the critical path.
                    ocp = spool.tile([128, 512], F32, tag="ocp")
                    nc.scalar.copy(ocp[0:65, :], pso[0:65, :])
                    rec = spool.tile([1, 512], F32, tag="rec")
                    nc.vector.reciprocal(rec, ocp[64:65, :])
                    bc = spool.tile([64, 512], F32, tag="bc")
                    nc.gpsimd.partition_broadcast(bc, rec)
                    if h == 1:
                        stg2 = spool.tile([64, 512], BF16, tag="stg2")
                        nc.vector.tensor_mul(stg2, ocp[0:64, :], bc)
                        nc.sync.dma_start(outT_sb[64:128, 0, qcs], stg2)
                    else:
                        nc.vector.tensor_mul(
                            outT_sb[0:64, slab, qcs], ocp[0:64, :], bc
                        )

                # projection for this q-chunk's 4 t-blocks
                for tb in range(4 * qc, 4 * qc + 4):
                    tbs = slice(tb * 128, (tb + 1) * 128)
                    ob = osbpool.tile([128, 768], F32, tag="osb")
                    for n0, nsz in ((0, 512), (512, 256)):
                        pp = psppool.tile([128, 512], F32, tag="pp", name="pp")
                        nc.tensor.matmul(
                            pp[:, :nsz],
                            lhsT=outT_sb[:, 0, tbs],
                            rhs=wp0_sb[:, n0:n0 + nsz],
                            start=True, stop=False,
                        )
                        nc.tensor.matmul(
                            pp[:, :nsz],
                            lhsT=outT_sb[:, 1, tbs],
                            rhs=wp1_sb[:, n0:n0 + nsz],
                            start=False, stop=True,
                        )
                        nc.vector.tensor_copy(ob[:, n0:n0 + nsz], pp[:, :nsz])
                    nc.sync.dma_start(out[tbs, :], ob)

    nc.compile()
    return nc


_NC_CACHE: dict[int, bass.Bass] = {}


def get_nc(seq_len: int) -> bass.Bass:
    if seq_len not in _NC_CACHE:
        _NC_CACHE[seq_len] = build_nc(seq_len)
    return _NC_CACHE[seq_len]


def make_in_maps(x: np.ndarray, w_attn: np.ndarray, w_proj: np.ndarray):
    """Per-core input dicts. Core c: batch c//4, head group c%4 (3 heads)."""
    bf16 = ml_dtypes.bfloat16
    in_maps = []
    for c in range(N_CORES):
        b, hg = divmod(c, 4)
        q = w_attn[192 * hg: 192 * hg + 192]
        k = w_attn[768 + 192 * hg: 768 + 192 * hg + 192]
        v = w_attn[1536 + 192 * hg: 1536 + 192 * hg + 192]
        wqk = np.concatenate([q[0:128], k[0:128], q[128:192], k[128:192]], axis=0)
        in_maps.append({
            "xT": np.ascontiguousarray(x[b].T).astype(bf16),
            "wqkT": np.ascontiguousarray(wqk.T).astype(bf16),
            "wvT": np.ascontiguousarray(v.T).astype(bf16),
            "wpT": np.ascontiguousarray(
                w_proj[:, 192 * hg: 192 * hg + 192].T
            ).astype(bf16),
        })
    return in_maps


def run_on_cores(x, w_attn, w_proj, trace: bool = False):
    from concourse.bass_utils import run_bass_kernel_spmd

    x = np.asarray(x, dtype=np.float32)
    w_attn = np.asarray(w_attn, dtype=np.float32)
    w_proj = np.asarray(w_proj, dtype=np.float32)
    nc = get_nc(x.shape[1])
    in_maps = make_in_maps(x, w_attn, w_proj)
    res = run_bass_kernel_spmd(
        nc, in_maps, core_ids=list(range(N_CORES)), trace=trace
    )
    outs = [r["out"] for r in res.results]
    full = np.stack(
        [sum(outs[4 * b + hg] for hg in range(4)) for b in range(B)], axis=0
    )
    return full, res


def kernel(x, w_attn, w_proj):
    full, _ = run_on_cores(x, w_attn, w_proj, trace=False)
    return full
